# revision 3
# baseline (speedup 1.0000x reference)
"""ChebyKAN layer kernel for TRN2 (8 NeuronCores, SPMD data-parallel over B).

y[b,o] = sum_{i,d} cos(d*arccos(tanh(x[b,i]))) * C[i,o,d]
       = sum_d T_d(tanh(x)) @ C[:,:,d]      (Chebyshev recurrence, exact)

v3: bf16 matmul operands (weights converted on host; basis chain kept in
f32 on VectorE, converted per-degree to bf16 on ScalarE).  Per-k tiles
for x and weights so DMA completion deps are exact; x DMAs issued on the
ScalarE hardware DGE queue, weights on the sync queue (parallel
channels); y stores on the gpsimd software DGE so they never block
weight prefetch.  Per-bank staggered eviction in the last degree so the
next chunk's matmuls start as PSUM banks free.  Degree-0 term folded
into a host-precomputed bias row added at eviction.
"""
import numpy as np
import ml_dtypes
from contextlib import ExitStack

import concourse.bass as bass
import concourse.tile as tile
from concourse import bacc, mybir
from concourse.bass_utils import run_bass_kernel_spmd

F32 = mybir.dt.float32
BF16 = mybir.dt.bfloat16
TANH = mybir.ActivationFunctionType.Tanh
COPY = mybir.ActivationFunctionType.Copy
MULT = mybir.AluOpType.mult
SUBTRACT = mybir.AluOpType.subtract
ADD = mybir.AluOpType.add

B, I, O, DEG = 16384, 1024, 1024, 8
N_CORES = 8
B_SHARD = B // N_CORES


def build_nc(I_=I, O_=O, b_shard=B_SHARD, b_chunk=512):
    """Build the per-core Bass program (SPMD: same program, sharded x)."""
    KT = I_ // 128          # contraction chunks
    MT = b_chunk // 128     # output-row tiles per chunk (PSUM partition dim)
    OHT = O_ // 512         # output-col halves per chunk (PSUM free dim)
    n_chunks = b_shard // b_chunk
    FD = KT * b_chunk       # free dim of basis tiles (k-major concat)
    assert MT * OHT <= 8

    nc = bacc.Bacc("TRN2", target_bir_lowering=False, debug=False)
    xT = nc.dram_tensor("xT", [I_, b_shard], F32, kind="ExternalInput").ap()
    w = nc.dram_tensor("w", [DEG, I_, O_], BF16, kind="ExternalInput").ap()
    biasrep = nc.dram_tensor("biasrep", [128, O_], F32, kind="ExternalInput").ap()
    y = nc.dram_tensor("y", [b_shard, O_], F32, kind="ExternalOutput").ap()

    with tile.TileContext(nc) as tc, ExitStack() as ctx:
        const_pool = ctx.enter_context(tc.tile_pool(name="const", bufs=1))
        x_pool = ctx.enter_context(tc.tile_pool(name="x", bufs=2))
        chain_pool = ctx.enter_context(tc.tile_pool(name="chain", bufs=1))
        bb_pool = ctx.enter_context(tc.tile_pool(name="bb", bufs=4))
        w_pool = ctx.enter_context(tc.tile_pool(name="w", bufs=3))
        stage_pool = ctx.enter_context(tc.tile_pool(name="stage", bufs=1))
        psum_pool = ctx.enter_context(tc.tile_pool(name="psum", bufs=1, space="PSUM"))

        def load_x(c):
            """Per-k x tiles, DMA'd on the ScalarE hardware queue."""
            xs = []
            for k in range(KT):
                xk = x_pool.tile([128, b_chunk], F32, tag=f"x{k}",
                                 name=f"x{k}_c{c}")
                nc.scalar.dma_start(
                    out=xk[:],
                    in_=xT[k * 128:(k + 1) * 128,
                           c * b_chunk:(c + 1) * b_chunk])
                xs.append(xk)
            return xs

        def load_w(c, d):
            """Per-k weight tiles for degree d on the sync queue."""
            ws = []
            for k in range(KT):
                wk = w_pool.tile([128, O_], BF16, tag=f"w{k}",
                                 name=f"w{d}k{k}_c{c}")
                nc.sync.dma_start(out=wk[:],
                                  in_=w[d - 1, k * 128:(k + 1) * 128, :])
                ws.append(wk)
            return ws

        x_next = load_x(0)
        w_next = load_w(0, 1)
        bias_t = const_pool.tile([128, O_], F32, tag="biasrep")
        nc.sync.dma_start(out=bias_t[:], in_=biasrep)

        NQ = 4                    # convert split granularity
        QW = FD // NQ             # columns per convert slice

        for c in range(n_chunks):
            b0 = c * b_chunk
            x_t = x_next

            # f32 recurrence chain tiles (single-buffered; WAR deps keep it
            # correct — last chain reads land early in each chunk's MM phase)
            t1 = chain_pool.tile([128, FD], F32, tag="t1", name=f"t1_c{c}")
            rings = [chain_pool.tile([128, FD], F32, tag=f"r{r}", name=f"r{r}_c{c}")
                     for r in range(3)]
            p_t = chain_pool.tile([128, FD], F32, tag="p", name=f"p_c{c}")

            # tanh + bf16 copy per k-slice: d=1 matmuls start after the
            # first slice instead of after the whole chunk's tanh
            t1b = bb_pool.tile([128, FD], BF16, tag="bb", name=f"t1b_c{c}")
            for k in range(KT):
                sl = slice(k * b_chunk, (k + 1) * b_chunk)
                nc.scalar.activation(t1[:, sl], x_t[k][:], TANH)
                nc.scalar.activation(t1b[:, sl], t1[:, sl], COPY)

            ps = [[psum_pool.tile([128, 512], F32, tag=f"ps{m}_{oh}",
                                  name=f"ps{m}_{oh}_c{c}")
                   for oh in range(OHT)] for m in range(MT)]

            t_prev2, t_prev1 = None, t1
            for d in range(1, DEG + 1):
                if d == 1:
                    tb = t1b
                    w_t = w_next
                else:
                    cur = rings[(d - 2) % 3]
                    if d == 2:
                        nc.vector.tensor_tensor(p_t[:], t1[:], t1[:], MULT)
                        nc.vector.tensor_scalar(cur[:], p_t[:], 2.0, -1.0,
                                                MULT, ADD)
                    else:
                        nc.vector.tensor_tensor(p_t[:], t1[:], t_prev1[:], MULT)
                        nc.vector.scalar_tensor_tensor(
                            cur[:], p_t[:], 2.0, t_prev2[:], MULT, SUBTRACT)
                    tb = bb_pool.tile([128, FD], BF16, tag="bb",
                                      name=f"tb{d}_c{c}")
                    for q in range(NQ):
                        qs = slice(q * QW, (q + 1) * QW)
                        nc.scalar.activation(tb[:, qs], cur[:, qs], COPY)
                    t_prev2, t_prev1 = t_prev1, cur
                    w_t = load_w(c, d)

                if d < DEG:
                    for k in range(KT):
                        for m in range(MT):
                            lhsT = tb[:, k * b_chunk + m * 128:
                                      k * b_chunk + (m + 1) * 128]
                            for oh in range(OHT):
                                nc.tensor.matmul(
                                    ps[m][oh][:], lhsT,
                                    w_t[k][:, oh * 512:(oh + 1) * 512],
                                    start=(d == 1 and k == 0), stop=False)
                else:
                    # prefetch next chunk's x + first-degree weights before
                    # the final MM block
                    if c + 1 < n_chunks:
                        x_next = load_x(c + 1)
                        w_next = load_w(c + 1, 1)
                    # last degree: per-bank k-contiguous accumulation, evict
                    # each bank the moment it completes
                    stage = stage_pool.tile([128, MT * 512], F32, tag="stage",
                                            name=f"st_c{c}")
                    for m in range(MT):
                        for oh in range(OHT):
                            for k in range(KT):
                                lhsT = tb[:, k * b_chunk + m * 128:
                                          k * b_chunk + (m + 1) * 128]
                                nc.tensor.matmul(
                                    ps[m][oh][:], lhsT,
                                    w_t[k][:, oh * 512:(oh + 1) * 512],
                                    start=False, stop=(k == KT - 1))
                            ssl = stage[:, m * 512:(m + 1) * 512]
                            nc.vector.tensor_tensor(
                                ssl, ps[m][oh][:],
                                bias_t[:, oh * 512:(oh + 1) * 512], ADD)
                            nc.gpsimd.dma_start(
                                out=y[b0 + m * 128: b0 + (m + 1) * 128,
                                      oh * 512:(oh + 1) * 512],
                                in_=ssl)
    nc.compile()
    return nc


_NC_CACHE = {}


def _install_ntff_hook():
    """Provide antenv.axon_hooks (missing in this image) so trace=True works."""
    import sys
    import types
    if "antenv.axon_hooks" in sys.modules:
        return
    hook = None
    try:
        from trn_agent_boot.trn_boot import _ntff_profile_via_ctypes
        hook = _ntff_profile_via_ctypes("/opt/axon/libaxon_pjrt.so")
    except Exception:
        pass
    mod = types.ModuleType("antenv.axon_hooks")
    mod.get_axon_ntff_profile_hook = lambda: hook
    sys.modules["antenv.axon_hooks"] = mod
    # no remote artifact bucket in this container
    import concourse.bass_utils as _bu
    _bu.upload_artifacts = lambda tmpdir: tmpdir


def _prep_inputs(x, cheby_coeffs, b_shard=B_SHARD, n_cores=N_CORES):
    coeffs = np.asarray(cheby_coeffs, dtype=np.float32)
    wperm = np.ascontiguousarray(
        np.moveaxis(coeffs[:, :, 1:], 2, 0)).astype(ml_dtypes.bfloat16)
    bias = coeffs[:, :, 0].astype(np.float64).sum(axis=0).astype(np.float32)
    biasrep = np.ascontiguousarray(
        np.broadcast_to(bias, (128, coeffs.shape[1])))
    xT = np.asarray(x, dtype=np.float32).T  # (I, B)
    in_maps = []
    for c in range(n_cores):
        in_maps.append({
            "xT": np.ascontiguousarray(xT[:, c * b_shard:(c + 1) * b_shard]),
            "w": wperm,
            "biasrep": biasrep,
        })
    return in_maps


def kernel(x: np.ndarray, cheby_coeffs: np.ndarray, _trace: bool = False):
    assert x.shape == (B, I) and cheby_coeffs.shape == (I, O, DEG + 1)
    if _trace:
        _install_ntff_hook()
    if "nc" not in _NC_CACHE:
        _NC_CACHE["nc"] = build_nc()
    nc = _NC_CACHE["nc"]

    in_maps = _prep_inputs(x, cheby_coeffs)
    res = run_bass_kernel_spmd(nc, in_maps, list(range(N_CORES)), trace=_trace)
    out = np.concatenate([res.results[c]["y"] for c in range(N_CORES)], axis=0)
    if _trace:
        return out, res
    return out


# revision 6
# speedup vs baseline: 1.0227x; 1.0227x over previous
"""ChebyKAN layer kernel for TRN2 (8 NeuronCores, SPMD data-parallel over B).

y[b,o] = sum_{i,d} cos(d*arccos(tanh(x[b,i]))) * C[i,o,d]
       = sum_d T_d(tanh(x)) @ C[:,:,d]      (Chebyshev recurrence, exact)

v3: bf16 matmul operands (weights converted on host; basis chain kept in
f32 on VectorE, converted per-degree to bf16 on ScalarE).  Per-k tiles
for x and weights so DMA completion deps are exact; x DMAs issued on the
ScalarE hardware DGE queue, weights on the sync queue (parallel
channels); y stores on the gpsimd software DGE so they never block
weight prefetch.  Per-bank staggered eviction in the last degree so the
next chunk's matmuls start as PSUM banks free.  Degree-0 term folded
into a host-precomputed bias row added at eviction.
"""
import numpy as np
import ml_dtypes
from contextlib import ExitStack

import concourse.bass as bass
import concourse.tile as tile
from concourse import bacc, mybir
from concourse.bass_utils import run_bass_kernel_spmd

F32 = mybir.dt.float32
BF16 = mybir.dt.bfloat16
TANH = mybir.ActivationFunctionType.Tanh
COPY = mybir.ActivationFunctionType.Copy
MULT = mybir.AluOpType.mult
SUBTRACT = mybir.AluOpType.subtract
ADD = mybir.AluOpType.add

B, I, O, DEG = 16384, 1024, 1024, 8
N_CORES = 8
B_SHARD = B // N_CORES


def build_nc(I_=I, O_=O, b_shard=B_SHARD, b_chunk=512):
    """Build the per-core Bass program (SPMD: same program, sharded x)."""
    KT = I_ // 128          # contraction chunks
    MT = b_chunk // 128     # output-row tiles per chunk (PSUM partition dim)
    OHT = O_ // 512         # output-col halves per chunk (PSUM free dim)
    n_chunks = b_shard // b_chunk
    FD = KT * b_chunk       # free dim of basis tiles (k-major concat)
    assert MT * OHT <= 8

    nc = bacc.Bacc("TRN2", target_bir_lowering=False, debug=False)
    xT = nc.dram_tensor("xT", [I_, b_shard], F32, kind="ExternalInput").ap()
    w = nc.dram_tensor("w", [DEG, I_, O_], BF16, kind="ExternalInput").ap()
    biasrep = nc.dram_tensor("biasrep", [128, O_], F32, kind="ExternalInput").ap()
    y = nc.dram_tensor("y", [b_shard, O_], F32, kind="ExternalOutput").ap()

    with tile.TileContext(nc) as tc, ExitStack() as ctx:
        const_pool = ctx.enter_context(tc.tile_pool(name="const", bufs=1))
        x_pool = ctx.enter_context(tc.tile_pool(name="x", bufs=2))
        chain_pool = ctx.enter_context(tc.tile_pool(name="chain", bufs=1))
        bb_pool = ctx.enter_context(tc.tile_pool(name="bb", bufs=4))
        w_pool = ctx.enter_context(tc.tile_pool(name="w", bufs=3))
        stage_pool = ctx.enter_context(tc.tile_pool(name="stage", bufs=1))
        psum_pool = ctx.enter_context(tc.tile_pool(name="psum", bufs=1, space="PSUM"))

        def load_w(c, d):
            """Per-k weight tiles for degree d on the sync queue."""
            ws = []
            for k in range(KT):
                wk = w_pool.tile([128, O_], BF16, tag=f"w{k}",
                                 name=f"w{d}k{k}_c{c}")
                nc.sync.dma_start(out=wk[:],
                                  in_=w[d - 1, k * 128:(k + 1) * 128, :])
                ws.append(wk)
            return ws

        def load_xw(c):
            """Interleave per-k x DMAs with degree-1 weight DMAs on the sync
            queue so both the tanh chain and the first matmul's weights land
            with minimal latency."""
            xs, ws = [], []
            for k in range(KT):
                xk = x_pool.tile([128, b_chunk], F32, tag=f"x{k}",
                                 name=f"x{k}_c{c}")
                nc.sync.dma_start(
                    out=xk[:],
                    in_=xT[k * 128:(k + 1) * 128,
                           c * b_chunk:(c + 1) * b_chunk])
                xs.append(xk)
                wk = w_pool.tile([128, O_], BF16, tag=f"w{k}",
                                 name=f"w1k{k}_c{c}")
                nc.sync.dma_start(out=wk[:],
                                  in_=w[0, k * 128:(k + 1) * 128, :])
                ws.append(wk)
            return xs, ws

        x_next, w_next = load_xw(0)
        bias_t = const_pool.tile([128, O_], F32, tag="biasrep")
        nc.gpsimd.dma_start(out=bias_t[:], in_=biasrep)

        NQ = 4                    # convert split granularity
        QW = FD // NQ             # columns per convert slice

        for c in range(n_chunks):
            b0 = c * b_chunk
            x_t = x_next

            # f32 recurrence chain tiles (single-buffered; WAR deps keep it
            # correct — last chain reads land early in each chunk's MM phase)
            t1 = chain_pool.tile([128, FD], F32, tag="t1", name=f"t1_c{c}")
            rings = [chain_pool.tile([128, FD], F32, tag=f"r{r}", name=f"r{r}_c{c}")
                     for r in range(3)]
            p_t = chain_pool.tile([128, FD], F32, tag="p", name=f"p_c{c}")

            # tanh + bf16 copy per k-slice: d=1 matmuls start after the
            # first slice instead of after the whole chunk's tanh
            t1b = bb_pool.tile([128, FD], BF16, tag="bb", name=f"t1b_c{c}")
            for k in range(KT):
                sl = slice(k * b_chunk, (k + 1) * b_chunk)
                nc.scalar.activation(t1[:, sl], x_t[k][:], TANH)
                nc.scalar.activation(t1b[:, sl], t1[:, sl], COPY)

            ps = [[psum_pool.tile([128, 512], F32, tag=f"ps{m}_{oh}",
                                  name=f"ps{m}_{oh}_c{c}")
                   for oh in range(OHT)] for m in range(MT)]

            t_prev2, t_prev1 = None, t1
            for d in range(1, DEG + 1):
                if d == 1:
                    tb = t1b
                    w_t = w_next
                else:
                    cur = rings[(d - 2) % 3]
                    if d == 2:
                        # halves: lets the first cv2 slice start as soon as
                        # the first half of tanh is done
                        for h in range(2):
                            hs = slice(h * (FD // 2), (h + 1) * (FD // 2))
                            nc.vector.tensor_tensor(p_t[:, hs], t1[:, hs],
                                                    t1[:, hs], MULT)
                            nc.vector.tensor_scalar(cur[:, hs], p_t[:, hs],
                                                    2.0, -1.0, MULT, ADD)
                    else:
                        nc.vector.tensor_tensor(p_t[:], t1[:], t_prev1[:], MULT)
                        nc.vector.scalar_tensor_tensor(
                            cur[:], p_t[:], 2.0, t_prev2[:], MULT, SUBTRACT)
                    tb = bb_pool.tile([128, FD], BF16, tag="bb",
                                      name=f"tb{d}_c{c}")
                    for q in range(NQ):
                        qs = slice(q * QW, (q + 1) * QW)
                        nc.scalar.activation(tb[:, qs], cur[:, qs], COPY)
                    t_prev2, t_prev1 = t_prev1, cur
                    w_t = load_w(c, d)

                if d < DEG:
                    for k in range(KT):
                        for m in range(MT):
                            lhsT = tb[:, k * b_chunk + m * 128:
                                      k * b_chunk + (m + 1) * 128]
                            for oh in range(OHT):
                                nc.tensor.matmul(
                                    ps[m][oh][:], lhsT,
                                    w_t[k][:, oh * 512:(oh + 1) * 512],
                                    start=(d == 1 and k == 0), stop=False)
                else:
                    # prefetch next chunk's x + first-degree weights before
                    # the final MM block
                    if c + 1 < n_chunks:
                        x_next, w_next = load_xw(c + 1)
                    # last degree: per-bank k-contiguous accumulation, evict
                    # each bank the moment it completes
                    stage = stage_pool.tile([128, MT * 512], F32, tag="stage",
                                            name=f"st_c{c}")
                    for m in range(MT):
                        for oh in range(OHT):
                            for k in range(KT):
                                lhsT = tb[:, k * b_chunk + m * 128:
                                          k * b_chunk + (m + 1) * 128]
                                nc.tensor.matmul(
                                    ps[m][oh][:], lhsT,
                                    w_t[k][:, oh * 512:(oh + 1) * 512],
                                    start=False, stop=(k == KT - 1))
                            ssl = stage[:, m * 512:(m + 1) * 512]
                            nc.vector.tensor_tensor(
                                ssl, ps[m][oh][:],
                                bias_t[:, oh * 512:(oh + 1) * 512], ADD)
                            nc.gpsimd.dma_start(
                                out=y[b0 + m * 128: b0 + (m + 1) * 128,
                                      oh * 512:(oh + 1) * 512],
                                in_=ssl)
    nc.compile()
    return nc


_NC_CACHE = {}


def _install_ntff_hook():
    """Provide antenv.axon_hooks (missing in this image) so trace=True works."""
    import sys
    import types
    if "antenv.axon_hooks" in sys.modules:
        return
    hook = None
    try:
        from trn_agent_boot.trn_boot import _ntff_profile_via_ctypes
        hook = _ntff_profile_via_ctypes("/opt/axon/libaxon_pjrt.so")
    except Exception:
        pass
    mod = types.ModuleType("antenv.axon_hooks")
    mod.get_axon_ntff_profile_hook = lambda: hook
    sys.modules["antenv.axon_hooks"] = mod
    # no remote artifact bucket in this container
    import concourse.bass_utils as _bu
    _bu.upload_artifacts = lambda tmpdir: tmpdir


def _prep_inputs(x, cheby_coeffs, b_shard=B_SHARD, n_cores=N_CORES):
    coeffs = np.asarray(cheby_coeffs, dtype=np.float32)
    wperm = np.ascontiguousarray(
        np.moveaxis(coeffs[:, :, 1:], 2, 0)).astype(ml_dtypes.bfloat16)
    bias = coeffs[:, :, 0].astype(np.float64).sum(axis=0).astype(np.float32)
    biasrep = np.ascontiguousarray(
        np.broadcast_to(bias, (128, coeffs.shape[1])))
    xT = np.asarray(x, dtype=np.float32).T  # (I, B)
    in_maps = []
    for c in range(n_cores):
        in_maps.append({
            "xT": np.ascontiguousarray(xT[:, c * b_shard:(c + 1) * b_shard]),
            "w": wperm,
            "biasrep": biasrep,
        })
    return in_maps


def kernel(x: np.ndarray, cheby_coeffs: np.ndarray, _trace: bool = False):
    assert x.shape == (B, I) and cheby_coeffs.shape == (I, O, DEG + 1)
    if _trace:
        _install_ntff_hook()
    if "nc" not in _NC_CACHE:
        _NC_CACHE["nc"] = build_nc()
    nc = _NC_CACHE["nc"]

    in_maps = _prep_inputs(x, cheby_coeffs)
    res = run_bass_kernel_spmd(nc, in_maps, list(range(N_CORES)), trace=_trace)
    out = np.concatenate([res.results[c]["y"] for c in range(N_CORES)], axis=0)
    if _trace:
        return out, res
    return out
